# revision 17
# baseline (speedup 1.0000x reference)
"""Trainium2 Bass kernel for nn_ConvSplitAttn_49065706390044.

Reference computation (input x: (B*N, D, LT) = (512, 64, 128) fp32):
  qkv = conv1d(groupnorm(x), w_qkv)              # k=3, pad=1
  q,k,v = split-rearrange to (B*H, N*S, D*L)
  attn = 1/cdist(q, k); out = attn @ v           # per (B*H) batch
  x = x + conv1d(out, w_merge)
  y = x + conv1d(swish(conv1d(groupnorm(x), w_ff1)), w_ff2)

Sharding: data-parallel over B (scenes): 8 cores x 2 scenes. Weights
replicated; each core gets a 64-row slice of x and returns the matching
64-row slice of y.

Per-core device program (scene-sequential). All activations are stored
duplicated across the two 64-partition SBUF halves so that the K=64
matmuls (convs with Cin=64, attention contractions with c=64) can be
emitted with alternating PE row-groups and run pairwise-concurrent on
the 128x128 array. Matmul inputs are fp32r (full-rate fp32 with reduced
mantissa). The q-branch conv weights are pre-scaled by -2 on the host so
the distance matrix D2^T = |k_j|^2 + |q_i|^2 - 2 q.k accumulates entirely
in PSUM: 16 l-step matmuls + squared-norm rows added via K=1 augmentation
matmuls (norm rows come from ones-matmuls over squared tiles). Then
attn = 1/sqrt(D2) via ACT Sqrt + DVE fast reciprocal. V^T and the
attn-output-to-conv-layout permutes run on PE in transpose mode, batched
per head-pair. GroupNorm gamma is folded into the following conv weights;
GroupNorm beta and all conv biases fold into the PSUM-evacuation ops
(beta boundary terms are asserted zero for this problem's inputs).
"""

import os
import sys

sys.path.insert(0, "/opt/trn_rl_repo")

import ml_dtypes
import numpy as np

import concourse.bacc as bacc
import concourse.bass as bass
import concourse.mybir as mybir
from concourse.tile import TileContext
from concourse import bass_utils

AF = mybir.ActivationFunctionType
ALU = mybir.AluOpType
F32 = mybir.dt.float32
F32R = mybir.dt.float32r
BF16 = mybir.dt.bfloat16

# problem dims
B, N, D, H, S, LT, DE = 16, 32, 64, 8, 8, 128, 256
L = LT // S          # 16
GROUPS = 8
EPS = 1e-5
NCORES = 8
SCENES_PER_CORE = B // NCORES   # 2
ROWS = SCENES_PER_CORE * N      # 64 rows of (B*N) per core
NB = 8                          # n-row blocks per scene (32 rows / 4)
RPB = N // NB                   # 4 rows per block
LTP = LT + 2                    # padded length

_CACHE: dict = {}
KPHASE = int(os.environ.get("KPHASE", "99"))


def _build():
    """Build the per-core Bass program (SPMD: same program on all 8 cores)."""
    nc = bacc.Bacc()

    x_d = nc.dram_tensor("x", [ROWS, D, LT], F32, kind="ExternalInput")
    wq_d = nc.dram_tensor("wq", [128, 3 * 1536], F32R, kind="ExternalInput")
    wm_d = nc.dram_tensor("wm", [128, 4 * 3 * 128], BF16, kind="ExternalInput")
    wf1_d = nc.dram_tensor("wf1", [128, 3 * 256], F32R, kind="ExternalInput")
    wf2_d = nc.dram_tensor("wf2", [128, 2 * 3 * 128], F32R, kind="ExternalInput")
    bias_d = nc.dram_tensor("biases", [128, 17], F32, kind="ExternalInput")
    gnm_d = nc.dram_tensor("gnm", [128, 128], F32, kind="ExternalInput")
    id_d = nc.dram_tensor("id128", [128, 128], F32R, kind="ExternalInput")
    ones_d = nc.dram_tensor("onesrow", [128, 256], F32R, kind="ExternalInput")
    onesb_d = nc.dram_tensor("onesb", [128, 256], BF16, kind="ExternalInput")
    idb_d = nc.dram_tensor("idb", [128, 128], BF16, kind="ExternalInput")
    y_d = nc.dram_tensor("y", [ROWS, D, LT], F32, kind="ExternalOutput")

    with TileContext(nc) as tc:
        with tc.tile_pool(name="sb", bufs=1) as pool, \
             tc.tile_pool(name="ps", bufs=1, space="PSUM") as psp:
            c = {}
            c["wq"] = pool.tile([128, 3 * 1536], F32R, tag="wq", name="wq_sb")
            c["wm"] = pool.tile([128, 4 * 3 * 128], BF16, tag="wm", name="wm_sb")
            c["wf1"] = pool.tile([128, 3 * 256], F32R, tag="wf1", name="wf1_sb")
            c["wf2"] = pool.tile([128, 2 * 3 * 128], F32R, tag="wf2", name="wf2_sb")
            c["bias"] = pool.tile([128, 17], F32, tag="bias", name="bias_sb")
            c["gnm"] = pool.tile([128, 128], F32, tag="gnm", name="gnm_sb")
            c["id"] = pool.tile([128, 128], F32R, tag="id", name="id_sb")
            c["ones"] = pool.tile([128, 256], F32R, tag="ones", name="ones_sb")
            c["onesb"] = pool.tile([128, 256], BF16, tag="onesb", name="onesb_sb")
            c["idb"] = pool.tile([128, 128], BF16, tag="idb", name="idb_sb")
            for key, src in (("wq", wq_d), ("wm", wm_d), ("wf1", wf1_d),
                             ("wf2", wf2_d), ("bias", bias_d), ("gnm", gnm_d),
                             ("id", id_d), ("ones", ones_d),
                             ("onesb", onesb_d), ("idb", idb_d)):
                nc.sync.dma_start(c[key][:, :], src[:, :])

            for sc in range(SCENES_PER_CORE):
                _scene(nc, sc, x_d, y_d, c, pool, psp)

    nc.compile()
    return nc


def _groupnorm(nc, pool, psp, gnm, xin, z_out, tag, eps_ap):
    """src_pad (128, N, LTP) fp32 with data duplicated on both partition
    halves -> z_out (128, N, LTP) fp32r normalized (no affine: gamma/beta
    are folded into the consuming conv)."""
    s12 = pool.tile([128, 64], F32, tag=f"s12{tag}")
    xsq = pool.tile([128, N, LT], F32, tag="scr")
    nc.scalar.activation(xsq[:, :, :], xin, AF.Square)
    nc.vector.tensor_reduce(s12[:, 0:32], xin, mybir.AxisListType.X, ALU.add)
    nc.vector.tensor_reduce(s12[:, 32:64], xsq[:, :, :], mybir.AxisListType.X,
                            ALU.add)
    pst = psp.tile([128, 64], F32, tag="psm", bufs=1)
    nc.tensor.matmul(pst[:, :], gnm[:, :], s12[:, :], start=True, stop=True)
    st = pool.tile([128, 64], F32, tag=f"st{tag}")
    nc.vector.tensor_copy(st[:, :], pst[:, :])
    mu = st[:, 0:32]
    m2 = st[:, 32:64]
    mu2 = pool.tile([128, 32], F32, tag=f"mu2{tag}")
    var = pool.tile([128, 32], F32, tag=f"var{tag}")
    rs = pool.tile([128, 32], F32, tag=f"rs{tag}")
    bb = pool.tile([128, 32], F32, tag=f"bb{tag}")
    nc.vector.tensor_tensor(mu2[:, :], mu, mu, ALU.mult)
    nc.vector.tensor_tensor(var[:, :], m2, mu2[:, :], ALU.subtract)
    nc.scalar.activation(var[:, :], var[:, :], AF.Sqrt, bias=eps_ap)
    nc.vector.reciprocal(rs[:, :], var[:, :])
    nc.vector.scalar_tensor_tensor(bb[:, :], mu, -1.0, rs[:, :],
                                   ALU.mult, ALU.mult)
    rs_b = rs[:, :].unsqueeze(2).broadcast_to([128, 32, 128])
    bb_b = bb[:, :].unsqueeze(2).broadcast_to([128, 32, 128])
    t = pool.tile([128, N, LT], F32, tag="scr")
    nc.vector.tensor_tensor(t[:, :, :], xin, rs_b, ALU.mult)
    nc.vector.tensor_tensor(z_out[:, :, 1:129], t[:, :, :], bb_b, ALU.add)


def _conv_k64_pair(nc, chains):
    """chains: list of (psum, w_sb, w_off, z_pad, nb, rg). Emits the 3
    shifted K=64 matmuls of each chain interleaved; each chain sticks to
    one PE row-group and its own PSUM bank (row-groups may not share an
    accumulating bank), so chains at rg=0/rg=64 run pairwise-concurrent."""
    for k in range(3):
        for psum, w_sb, w_off, z_pad, nb, rg in chains:
            off = w_off(k)
            nc.tensor.matmul(
                psum[:, :, :], w_sb[rg:rg + 64, off:off + 128],
                z_pad[rg:rg + 64, nb * RPB:(nb + 1) * RPB, k:k + 128],
                start=(k == 0), stop=(k == 2), tile_position=(rg, 0))


def _scene(nc, sc, x_d, y_d, c, pool, psp):
    bias = c["bias"]
    # ---- load x (duplicated halves, padded) ----
    x2 = pool.tile([128, N, LT], F32, tag="mcx0")
    xsrc = x_d[sc * N:(sc + 1) * N, :, :].rearrange("n c t -> c n t")
    nc.sync.dma_start(x2[0:64, :, :], xsrc)
    nc.sync.dma_start(x2[64:128, :, :], xsrc)

    # ---- GN1 -> z2 ----
    z2 = pool.tile([128, N, LTP], F32R, tag="z")
    nc.gpsimd.memset(z2[:, :, :].bitcast(mybir.dt.uint32), 0)
    _groupnorm(nc, pool, psp, c["gnm"], x2[:, :, :], z2, "g1",
               c["bias"][:, 16:17])

    # merge-conv input: 4 chunks of ((2 heads, 64 chan), n, lt padded).
    # chunk 0 reuses x2's slot (x2 is dead after the GN1 normalize pass;
    # the residual re-loads x from DRAM later).
    mc_in = [pool.tile([128, N, LTP], BF16, tag=f"mcx{kc}" if kc == 0
                       else f"mc{kc}", name=f"mc_in{kc}") for kc in range(4)]
    for kc in range(4):
        nc.gpsimd.memset(mc_in[kc][:, :, :].bitcast(mybir.dt.uint16), 0)

    for hp in range(4 if KPHASE >= 2 else 0):
        # ---- qkv conv for this head pair ----
        qkv = []
        for t in range(3):
            tagn = ("qbuf", "kbuf", "vy")[t]
            if t == 0:
                dst = pool.tile([128, N, LT], F32R, tag=tagn, name=f"qkv{t}")
            else:
                # (c, l, n, s) layout so per-(jc, l) lhsT slices are contiguous
                dst = pool.tile([128, L, N, S], F32R, tag=tagn, name=f"qkv{t}")
            m = t * 4 + hp
            woff = lambda k: k * 1536 + m * 128
            for nb0 in range(0, NB, 2):
                pqs = [psp.tile([128, RPB, LT], F32, tag="big", bufs=3,
                                name=f"pq{i}") for i in range(2)]
                _conv_k64_pair(nc, [
                    (pqs[0], c["wq"], woff, z2, nb0, 0),
                    (pqs[1], c["wq"], woff, z2, nb0 + 1, 64)])
                for i, pq in enumerate(pqs):
                    nb = nb0 + i
                    if t == 0:
                        out_ap = dst[:, nb * RPB:(nb + 1) * RPB, :]
                        src_ap = pq[:, :, :]
                    else:
                        out_ap = dst[:, :, nb * RPB:(nb + 1) * RPB, :]
                        src_ap = pq[:, :, :].rearrange("p n (l s) -> p l n s",
                                                       l=L)
                    if i == 0:
                        nc.scalar.activation(out_ap, src_ap, AF.Identity,
                                             bias=bias[:, m:m + 1])
                    else:
                        nc.vector.tensor_scalar(out_ap, src_ap,
                                                bias[:, m:m + 1], None,
                                                ALU.add)
            qkv.append(dst)
        qp, kp, vp = qkv  # q-branch pre-scaled by -2 (host)

        if KPHASE < 3:
            continue
        # ---- squared tiles: qsq = 0.25*q'^2 (= q^2), ksq = k^2 ----
        qsq = pool.tile([128, N, LT], BF16, tag="scr", name="sqq")
        nc.vector.scalar_tensor_tensor(qsq[:, :, :], qp[:, :, :], 0.25,
                                       qp[:, :, :], ALU.mult, ALU.mult)
        ksq = pool.tile([128, L, N, S], BF16, tag="ksq", name="sqk")
        nc.scalar.activation(ksq[:, :, :, :], kp[:, :, :, :], AF.Square)

        if KPHASE < 4:
            continue
        # ---- D2^T = -2 k.q + |q_i|^2 + |k_j|^2, all in PSUM ----
        attn = {}
        for jc in range(2):
            pd = {p: psp.tile([128, 256], F32, tag="d2", bufs=2,
                               name=f"pd{p}") for p in range(2)}
            for l in range(L):
                for p in range(2):
                    rg = p * 64
                    nc.tensor.matmul(
                        pd[p][:, :],
                        kp[rg:rg + 64, l, jc * 16:(jc + 1) * 16, :],
                        qp[rg:rg + 64, :, l * 8:(l + 1) * 8],
                        start=(l == 0), stop=False, tile_position=(rg, 0))
            for l in range(L):
                for p in range(2):
                    rg = p * 64
                    # + |q_i|^2 broadcast over j (ones^T @ qsq)
                    nc.tensor.matmul(
                        pd[p][:, :], c["onesb"][rg:rg + 64, 0:128],
                        qsq[rg:rg + 64, :, l * 8:(l + 1) * 8],
                        start=False, stop=False, tile_position=(rg, 0))
                    # + |k_j|^2 broadcast over i (ksq^T @ ones)
                    nc.tensor.matmul(
                        pd[p][:, :],
                        ksq[rg:rg + 64, l, jc * 16:(jc + 1) * 16, :],
                        c["onesb"][rg:rg + 64, 0:256],
                        start=False, stop=(l == L - 1),
                        tile_position=(rg, 0))
            for p in range(2):
                dist = pool.tile([128, 256], F32, tag=f"dist{p}")
                af = pool.tile([128, 256], F32, tag=f"af{p}")
                ar = pool.tile([128, 256], F32R, tag=f"ar{p}{jc}")
                nc.scalar.activation(dist[:, :], pd[p][:, :], AF.Sqrt)
                nc.vector.reciprocal_approx_fast(af[:, :], dist[:, :])
                nc.vector.tensor_copy(ar[:, :], af[:, :])
                attn[(p, jc)] = ar

        if KPHASE < 5:
            continue
        # ---- V^T via PE transposes (both heads at once) ----
        vT = [pool.tile([128, 2048], F32R, tag=f"vT{jc}", name=f"vT{jc}")
              for jc in range(2)]
        for jc in range(2):
            vTr = vT[jc].rearrange("p (h c l) -> p h l c", h=2, c=64, l=L)
            for lg in range(4):
                pvt = psp.tile([128, 4, 128], F32R, tag="bigr", bufs=2)
                for j in range(4):
                    l = lg * 4 + j
                    nc.tensor.transpose(
                        pvt[:, j, :],
                        vp[:, l, jc * 16:(jc + 1) * 16, :],
                        c["id"][:, :])
                dst = vTr[:, :, lg * 4:(lg + 1) * 4, :]
                src = pvt[:, :, :].rearrange("p l (h c) -> p h l c", h=2)
                if lg % 2 == 0:
                    nc.vector.tensor_copy(dst, src)
                else:
                    nc.scalar.activation(dst, src, AF.Copy)

        # ---- AV matmuls -> av_pair tiles (i-chunk part, (2h, d) free) ----
        av = [pool.tile([128, 2048], BF16, tag=f"av{ic}", name=f"av{ic}")
              for ic in range(2)]
        for p in range(2):
            for ic in range(2):
                for dh in range(2):
                    pav = psp.tile([128, 512], F32, tag="big", bufs=3)
                    o = p * 1024 + dh * 512
                    for jc in range(2):
                        nc.tensor.matmul(
                            pav[:, :],
                            attn[(p, jc)][:, ic * 128:(ic + 1) * 128],
                            vT[jc][:, o:o + 512],
                            start=(jc == 0), stop=(jc == 1))
                    # av free layout is (l, h, c): d = l*128 + h*64 + c
                    av4 = av[ic].rearrange("p (l h c) -> p h c l", h=2, c=64)
                    dst = av4[:, p, dh * 32:(dh + 1) * 32, :]
                    srcp = pav[:, :].rearrange("p (c l) -> p c l", c=32)
                    if dh == 0:
                        nc.vector.tensor_copy(dst, srcp)
                    else:
                        nc.scalar.activation(dst, srcp, AF.Copy)

        # ---- permute attn-out into merge-conv layout mc_in[hp] ----
        mcr = mc_in[hp][:, :, 1:129].rearrange("p n (l s) -> p l n s", l=L)
        for ic in range(2):
            avT = av[ic].rearrange("p (l h c) -> p l h c", h=2, c=64)
            for lg in range(4):
                pmc = psp.tile([128, 4, 128], BF16, tag="bigr", bufs=2,
                               name="pmc")
                for j in range(4):
                    l = lg * 4 + j
                    nc.tensor.transpose(pmc[:, j, :], avT[:, l, :, :],
                                        c["idb"][:, :])
                dst = mcr[:, lg * 4:(lg + 1) * 4, ic * 16:(ic + 1) * 16, :]
                src = pmc[:, :, :].rearrange("p l (n s) -> p l n s", n=16)
                if lg % 2 == 0:
                    nc.vector.tensor_copy(dst, src)
                else:
                    nc.scalar.activation(dst, src, AF.Copy)

    if KPHASE < 6:
        out_sb = pool.tile([64, N, LT], F32, tag="scr", name="outsb0")
        nc.vector.memset(out_sb[:, :, :], 0.0)
        ydst0 = y_d[sc * N:(sc + 1) * N, :, :].rearrange("n c t -> c n t")
        nc.sync.dma_start(ydst0, out_sb[:, :, :])
        return
    # ---- merge conv + residual -> y2p (padded, duplicated) ----
    xr = pool.tile([128, N, LT], F32, tag="scr")
    nc.sync.dma_start(xr[0:64, :, :], xsrc)
    nc.sync.dma_start(xr[64:128, :, :], xsrc)
    y2p = pool.tile([128, N, LTP], F32, tag="vy")
    for nb in range(NB):
        pm = psp.tile([128, RPB, LT], F32, tag="big", bufs=3)
        for kc in range(4):
            for k in range(3):
                o = (kc * 3 + k) * 128
                nc.tensor.matmul(
                    pm[:, :, :], c["wm"][:, o:o + 128],
                    mc_in[kc][:, nb * RPB:(nb + 1) * RPB, k:k + 128],
                    start=(kc == 0 and k == 0), stop=(kc == 3 and k == 2))
        nc.vector.scalar_tensor_tensor(
            y2p[:, nb * RPB:(nb + 1) * RPB, 1:129], pm[:, :, :],
            bias[:, 12:13], xr[:, nb * RPB:(nb + 1) * RPB, :],
            ALU.add, ALU.add)

    if KPHASE < 7:
        out_sb = pool.tile([64, N, LT], F32, tag="scr", name="outsb0")
        nc.vector.tensor_copy(out_sb[:, :, :], y2p[0:64, :, 1:129])
        ydst0 = y_d[sc * N:(sc + 1) * N, :, :].rearrange("n c t -> c n t")
        nc.sync.dma_start(ydst0, out_sb[:, :, :])
        return
    # ---- GN2 -> z3 (shares z2's slot) ----
    z3 = pool.tile([128, N, LTP], F32R, tag="z")
    nc.gpsimd.memset(z3[:, :, :].bitcast(mybir.dt.uint32), 0)
    _groupnorm(nc, pool, psp, c["gnm"], y2p[:, :, 1:129], z3, "g2",
               c["bias"][:, 16:17])

    # ---- FFN per n-block: ff1 (+swish) then ff2 (+residual) ----
    out_sb = pool.tile([64, N, LT], F32, tag="scr")
    for nb in range(NB):
        h2 = [pool.tile([128, RPB, LTP], F32R, tag=f"h2{mc}", bufs=2,
                        name=f"h2{mc}") for mc in range(2)]
        pfs = [psp.tile([128, RPB, LT], F32, tag="big", bufs=3,
                        name=f"pf{i}") for i in range(2)]
        for mc in range(2):
            nc.gpsimd.memset(h2[mc][:, :, :].bitcast(mybir.dt.uint32), 0)
        _conv_k64_pair(nc, [
            (pfs[0], c["wf1"], lambda k: k * 256, z3, nb, 0),
            (pfs[1], c["wf1"], lambda k: k * 256 + 128, z3, nb, 64)])
        for mc in range(2):
            pf = pfs[mc]
            sg = pool.tile([128, RPB, LT], F32, tag="sg", bufs=1, name="sg")
            nc.scalar.activation(sg[:, :, :], pf[:, :, :], AF.Sigmoid,
                                 bias=bias[:, 13 + mc:14 + mc])
            nc.vector.scalar_tensor_tensor(h2[mc][:, :, 1:129], pf[:, :, :],
                                           bias[:, 13 + mc:14 + mc],
                                           sg[:, :, :], ALU.add, ALU.mult)
        po = psp.tile([128, RPB, LT], F32, tag="big", bufs=3)
        for kc in range(2):
            for k in range(3):
                o = (kc * 3 + k) * 128
                nc.tensor.matmul(po[:, :, :], c["wf2"][:, o:o + 128],
                                 h2[kc][:, :, k:k + 128],
                                 start=(kc == 0 and k == 0),
                                 stop=(kc == 1 and k == 2))
        nc.vector.scalar_tensor_tensor(
            out_sb[:, nb * RPB:(nb + 1) * RPB, :], po[0:64, :, :],
            bias[0:64, 15:16], y2p[0:64, nb * RPB:(nb + 1) * RPB, 1:129],
            ALU.add, ALU.add)
    ydst = y_d[sc * N:(sc + 1) * N, :, :].rearrange("n c t -> c n t")
    nc.sync.dma_start(ydst, out_sb[:, :, :])


# ---------------------------------------------------------------------------
# host side
# ---------------------------------------------------------------------------

def _prep_consts(inputs):
    f = np.float32
    gn1_g = np.asarray(inputs["gn1_g"], f)
    gn1_b = np.asarray(inputs["gn1_b"], f)
    gn2_g = np.asarray(inputs["gn2_g"], f)
    gn2_b = np.asarray(inputs["gn2_b"], f)
    w_qkv = np.asarray(inputs["w_qkv"], f)      # (1536, 64, 3)
    b_qkv = np.asarray(inputs["b_qkv"], f)
    w_merge = np.asarray(inputs["w_merge"], f)  # (64, 512, 3)
    b_merge = np.asarray(inputs["b_merge"], f)
    w_ff1 = np.asarray(inputs["w_ff1"], f)      # (256, 64, 3)
    b_ff1 = np.asarray(inputs["b_ff1"], f)
    w_ff2 = np.asarray(inputs["w_ff2"], f)      # (64, 256, 3)
    b_ff2 = np.asarray(inputs["b_ff2"], f)

    co_scale = np.ones((1536,), f)
    co_scale[0:512] = -2.0  # q-branch
    wq_eff = w_qkv * gn1_g[None, :, None] * co_scale[:, None, None]
    wqT = np.ascontiguousarray(wq_eff.transpose(1, 2, 0)).reshape(64, 3 * 1536)
    wq_host = np.concatenate([wqT, wqT], axis=0)  # (128, 4608)

    Ck = np.einsum("oik,i->ok", w_qkv, gn1_b) * co_scale[:, None]
    assert abs(Ck).max() == 0.0, "nonzero gn1 beta needs boundary fixups"
    bq_eff = b_qkv * co_scale + Ck.sum(1)

    wmT = np.ascontiguousarray(w_merge.transpose(1, 2, 0))  # (512, 3, 64)
    wm_host = np.concatenate([wmT, wmT], axis=2)            # (512, 3, 128)
    wm_host = wm_host.reshape(4, 128, 3 * 128).transpose(1, 0, 2) \
                     .reshape(128, 4 * 3 * 128)

    wf1_eff = w_ff1 * gn2_g[None, :, None]
    Ck2 = np.einsum("oik,i->ok", w_ff1, gn2_b)
    assert abs(Ck2).max() == 0.0, "nonzero gn2 beta needs boundary fixups"
    bf1_eff = b_ff1 + Ck2.sum(1)
    wf1T = np.ascontiguousarray(wf1_eff.transpose(1, 2, 0)).reshape(64, 768)
    wf1_host = np.concatenate([wf1T, wf1T], axis=0)  # (128, 768)

    wf2T = np.ascontiguousarray(w_ff2.transpose(1, 2, 0))  # (256, 3, 64)
    wf2_host = np.concatenate([wf2T, wf2T], axis=2)        # (256, 3, 128)
    wf2_host = wf2_host.reshape(2, 128, 3 * 128).transpose(1, 0, 2) \
                       .reshape(128, 2 * 3 * 128)

    biases = np.zeros((128, 17), f)
    biases[:, 16] = EPS
    biases[:, 0:12] = bq_eff.reshape(12, 128).T
    biases[:, 12] = np.concatenate([b_merge, b_merge])
    biases[:, 13:15] = bf1_eff.reshape(2, 128).T
    biases[:, 15] = np.concatenate([b_ff2, b_ff2])

    p = np.arange(128)
    gnm = ((p[:, None] % 64) // 8 == (p[None, :] % 64) // 8).astype(f)
    gnm /= (D // GROUPS) * LT * 2  # mean over group, duplicated halves

    return {
        "wq": np.ascontiguousarray(wq_host),
        "wm": np.ascontiguousarray(wm_host).astype(ml_dtypes.bfloat16),
        "wf1": np.ascontiguousarray(wf1_host),
        "wf2": np.ascontiguousarray(wf2_host),
        "biases": biases,
        "gnm": gnm,
        "id128": np.eye(128, dtype=f),
        "onesrow": np.ones((128, 256), f),
        "onesb": np.ones((128, 256), ml_dtypes.bfloat16),
        "idb": np.eye(128, dtype=f).astype(ml_dtypes.bfloat16),
    }


def build_in_maps(inputs):
    x = np.ascontiguousarray(np.asarray(inputs["x"], np.float32))
    consts = _prep_consts(inputs)
    in_maps = []
    for core in range(NCORES):
        m = dict(consts)
        m["x"] = np.ascontiguousarray(x[core * ROWS:(core + 1) * ROWS])
        in_maps.append(m)
    return in_maps


def get_program():
    if "nc" not in _CACHE:
        _CACHE["nc"] = _build()
    return _CACHE["nc"]


def kernel(**inputs) -> np.ndarray:
    nc = get_program()
    in_maps = build_in_maps(inputs)
    res = bass_utils.run_bass_kernel_spmd(nc, in_maps,
                                          core_ids=list(range(NCORES)))
    out = np.concatenate([res.results[c]["y"] for c in range(NCORES)], axis=0)
    return out.astype(np.float32)


# revision 22
# speedup vs baseline: 1.2841x; 1.2841x over previous
"""Trainium2 Bass kernel for nn_ConvSplitAttn_49065706390044.

Reference computation (input x: (B*N, D, LT) = (512, 64, 128) fp32):
  qkv = conv1d(groupnorm(x), w_qkv)              # k=3, pad=1
  q,k,v = split-rearrange to (B*H, N*S, D*L)
  attn = 1/cdist(q, k); out = attn @ v           # per (B*H) batch
  x = x + conv1d(out, w_merge)
  y = x + conv1d(swish(conv1d(groupnorm(x), w_ff1)), w_ff2)

Sharding: data-parallel over B (scenes): 8 cores x 2 scenes. Weights
replicated; each core gets a 64-row slice of x and returns the matching
64-row slice of y.

Per-core device program (scene-sequential). All activations are stored
duplicated across the two 64-partition SBUF halves so that the K=64
matmuls (convs with Cin=64, attention contractions with c=64) can be
emitted with alternating PE row-groups and run pairwise-concurrent on
the 128x128 array. Matmul inputs are fp32r (full-rate fp32 with reduced
mantissa). The q-branch conv weights are pre-scaled by -2 on the host so
the distance matrix D2^T = |k_j|^2 + |q_i|^2 - 2 q.k accumulates entirely
in PSUM: 16 l-step matmuls + squared-norm rows added via K=1 augmentation
matmuls (norm rows come from ones-matmuls over squared tiles). Then
attn = 1/sqrt(D2) via ACT Sqrt + DVE fast reciprocal. V^T and the
attn-output-to-conv-layout permutes run on PE in transpose mode, batched
per head-pair. GroupNorm gamma is folded into the following conv weights;
GroupNorm beta and all conv biases fold into the PSUM-evacuation ops
(beta boundary terms are asserted zero for this problem's inputs).
"""

import os
import sys

sys.path.insert(0, "/opt/trn_rl_repo")

import ml_dtypes
import numpy as np

import concourse.bacc as bacc
import concourse.bass as bass
import concourse.mybir as mybir
from concourse.tile import TileContext
from concourse import bass_utils

AF = mybir.ActivationFunctionType
ALU = mybir.AluOpType
F32 = mybir.dt.float32
F32R = mybir.dt.float32r
BF16 = mybir.dt.bfloat16

# problem dims
B, N, D, H, S, LT, DE = 16, 32, 64, 8, 8, 128, 256
L = LT // S          # 16
GROUPS = 8
EPS = 1e-5
NCORES = 8
SCENES_PER_CORE = B // NCORES   # 2
ROWS = SCENES_PER_CORE * N      # 64 rows of (B*N) per core
NB = 8                          # n-row blocks per scene (32 rows / 4)
RPB = N // NB                   # 4 rows per block
LTP = LT + 2                    # padded length

_CACHE: dict = {}
KPHASE = int(os.environ.get("KPHASE", "99"))


def _build():
    """Build the per-core Bass program (SPMD: same program on all 8 cores)."""
    nc = bacc.Bacc()

    x_d = nc.dram_tensor("x", [D, ROWS, LT], F32, kind="ExternalInput")
    wq_d = nc.dram_tensor("wq", [128, 3 * 1536], F32R, kind="ExternalInput")
    wm_d = nc.dram_tensor("wm", [128, 4 * 3 * 128], BF16, kind="ExternalInput")
    wf1_d = nc.dram_tensor("wf1", [128, 3 * 256], F32R, kind="ExternalInput")
    wf2_d = nc.dram_tensor("wf2", [128, 2 * 3 * 128], F32R, kind="ExternalInput")
    bias_d = nc.dram_tensor("biases", [128, 17], F32, kind="ExternalInput")
    gnm_d = nc.dram_tensor("gnm", [128, 128], F32, kind="ExternalInput")
    onesb_d = nc.dram_tensor("onesb", [128, 256], BF16, kind="ExternalInput")
    idb_d = nc.dram_tensor("idb", [128, 128], BF16, kind="ExternalInput")
    y_d = nc.dram_tensor("y", [D, ROWS, LT], F32, kind="ExternalOutput")

    with TileContext(nc) as tc:
        with tc.tile_pool(name="sb", bufs=1) as pool, \
             tc.tile_pool(name="ps", bufs=1, space="PSUM") as psp:
            c = {}
            c["wq"] = pool.tile([128, 3 * 1536], F32R, tag="wq", name="wq_sb")
            c["wm"] = pool.tile([128, 4 * 3 * 128], BF16, tag="wm", name="wm_sb")
            c["wf1"] = pool.tile([128, 3 * 256], F32R, tag="wf1", name="wf1_sb")
            c["wf2"] = pool.tile([128, 2 * 3 * 128], F32R, tag="wf2", name="wf2_sb")
            c["bias"] = pool.tile([128, 17], F32, tag="bias", name="bias_sb")
            c["gnm"] = pool.tile([128, 128], F32, tag="gnm", name="gnm_sb")
            c["onesb"] = pool.tile([128, 256], BF16, tag="onesb", name="onesb_sb")
            c["idb"] = pool.tile([128, 128], BF16, tag="idb", name="idb_sb")
            for key, src in (("wq", wq_d), ("wm", wm_d), ("wf1", wf1_d),
                             ("wf2", wf2_d), ("bias", bias_d), ("gnm", gnm_d),
                             ("onesb", onesb_d), ("idb", idb_d)):
                nc.sync.dma_start(c[key][:, :], src[:, :])

            for sc in range(SCENES_PER_CORE):
                _scene(nc, sc, x_d, y_d, c, pool, psp)

    nc.compile()
    return nc


def _groupnorm(nc, pool, psp, gnm, xin, z_out, tag, eps_ap):
    """src_pad (128, N, LTP) fp32 with data duplicated on both partition
    halves -> z_out (128, N, LTP) fp32r normalized (no affine: gamma/beta
    are folded into the consuming conv)."""
    s12 = pool.tile([128, 64], F32, tag=f"s12{tag}")
    xsq = pool.tile([128, N, LT], F32, tag="scr")
    nc.scalar.activation(xsq[:, :, :], xin, AF.Square)
    nc.vector.tensor_reduce(s12[:, 0:32], xin, mybir.AxisListType.X, ALU.add)
    nc.vector.tensor_reduce(s12[:, 32:64], xsq[:, :, :], mybir.AxisListType.X,
                            ALU.add)
    pst = psp.tile([128, 256], F32, tag="d2", bufs=2)
    nc.tensor.matmul(pst[:, 0:64], gnm[:, :], s12[:, :], start=True, stop=True)
    st = pool.tile([128, 64], F32, tag=f"st{tag}")
    nc.vector.tensor_copy(st[:, :], pst[:, 0:64])
    mu = st[:, 0:32]
    m2 = st[:, 32:64]
    mu2 = pool.tile([128, 32], F32, tag=f"mu2{tag}")
    var = pool.tile([128, 32], F32, tag=f"var{tag}")
    rs = pool.tile([128, 32], F32, tag=f"rs{tag}")
    bb = pool.tile([128, 32], F32, tag=f"bb{tag}")
    nc.vector.tensor_tensor(mu2[:, :], mu, mu, ALU.mult)
    nc.vector.tensor_tensor(var[:, :], m2, mu2[:, :], ALU.subtract)
    nc.scalar.activation(var[:, :], var[:, :], AF.Sqrt, bias=eps_ap)
    nc.vector.reciprocal(rs[:, :], var[:, :])
    nc.vector.scalar_tensor_tensor(bb[:, :], mu, -1.0, rs[:, :],
                                   ALU.mult, ALU.mult)
    # normalize: z = x*rs + bb, one fused op per n-row (per-partition
    # scalars), split across ACT and DVE
    for n in range(N):
        zo = z_out[:, n, 1:129]
        xi = xin[:, n, :]
        if n % 2 == 0:
            nc.scalar.activation(zo, xi, AF.Identity, bias=bb[:, n:n + 1],
                                 scale=rs[:, n:n + 1])
        else:
            nc.vector.tensor_scalar(zo, xi, rs[:, n:n + 1], bb[:, n:n + 1],
                                    ALU.mult, ALU.add)


def _conv_k64_pair(nc, chains):
    """chains: list of (psum, w_sb, w_off, z_pad, nb, rg). Emits the 3
    shifted K=64 matmuls of each chain interleaved; each chain sticks to
    one PE row-group and its own PSUM bank (row-groups may not share an
    accumulating bank), so chains at rg=0/rg=64 run pairwise-concurrent."""
    for k in range(3):
        for psum, w_sb, w_off, z_pad, nb, rg in chains:
            off = w_off(k)
            nc.tensor.matmul(
                psum[:, :, :], w_sb[rg:rg + 64, off:off + 128],
                z_pad[rg:rg + 64, nb * RPB:(nb + 1) * RPB, k:k + 128],
                start=(k == 0), stop=(k == 2), tile_position=(rg, 0))


def _scene(nc, sc, x_d, y_d, c, pool, psp):
    bias = c["bias"]
    # ---- load x (duplicated halves, padded) ----
    x2 = pool.tile([128, N, LT], F32, tag="mcx0")
    xsrc = x_d[:, sc * N:(sc + 1) * N, :]
    nc.sync.dma_start(x2[0:64, :, :], xsrc)
    nc.sync.dma_start(x2[64:128, :, :], xsrc)

    # ---- GN1 -> z2 ----
    z2 = pool.tile([128, N, LTP], F32R, tag="z")
    nc.gpsimd.memset(z2[:, :, :].bitcast(mybir.dt.uint32), 0)
    _groupnorm(nc, pool, psp, c["gnm"], x2[:, :, :], z2, "g1",
               c["bias"][:, 16:17])

    # merge-conv input: 4 chunks of ((2 heads, 64 chan), n, lt padded).
    # chunk 0 reuses x2's slot (x2 is dead after the GN1 normalize pass;
    # the residual re-loads x from DRAM later).
    mc_in = [pool.tile([128, N, LTP], BF16, tag=f"mcx{kc}" if kc == 0
                       else f"mc{kc}", name=f"mc_in{kc}") for kc in range(4)]
    for kc in range(4):
        nc.gpsimd.memset(mc_in[kc][:, :, :].bitcast(mybir.dt.uint16), 0)

    for hp in range(4 if KPHASE >= 2 else 0):
        # ---- qkv conv for this head pair ----
        qkv = []
        for t in range(3):
            tagn = ("qbuf", "kbuf", "vy")[t]
            if t == 0:
                dst = pool.tile([128, N, LT], F32R, tag=tagn, name=f"qkv{t}")
            else:
                # (c, l, n, s) layout so per-(jc, l) lhsT slices are contiguous
                dt_t = F32R if t == 1 else BF16
                dst = pool.tile([128, L, N, S], dt_t, tag=tagn, name=f"qkv{t}")
            m = t * 4 + hp
            woff = lambda k: k * 1536 + m * 128
            for nb0 in range(0, NB, 2):
                pqs = [psp.tile([128, RPB, LT], F32, tag="big", bufs=4,
                                name=f"pq{i}") for i in range(2)]
                _conv_k64_pair(nc, [
                    (pqs[0], c["wq"], woff, z2, nb0, 0),
                    (pqs[1], c["wq"], woff, z2, nb0 + 1, 64)])
                for i, pq in enumerate(pqs):
                    nb = nb0 + i
                    if t == 0:
                        out_ap = dst[:, nb * RPB:(nb + 1) * RPB, :]
                        src_ap = pq[:, :, :]
                    else:
                        out_ap = dst[:, :, nb * RPB:(nb + 1) * RPB, :]
                        src_ap = pq[:, :, :].rearrange("p n (l s) -> p l n s",
                                                       l=L)
                    if i == 0:
                        nc.scalar.activation(out_ap, src_ap, AF.Identity,
                                             bias=bias[:, m:m + 1])
                    else:
                        nc.vector.tensor_scalar(out_ap, src_ap,
                                                bias[:, m:m + 1], None,
                                                ALU.add)
            qkv.append(dst)
        qp, kp, vp = qkv  # q-branch pre-scaled by -2 (host)

        if KPHASE < 3:
            continue
        # ---- squared tiles: qsq = 0.25*q'^2 (= q^2), ksq = k^2 ----
        qsq = pool.tile([128, N, LT], BF16, tag="scr", name="sqq")
        nc.vector.scalar_tensor_tensor(qsq[:, :, :], qp[:, :, :], 0.25,
                                       qp[:, :, :], ALU.mult, ALU.mult)
        ksq = pool.tile([128, L, N, S], BF16, tag="ksq", name="sqk")
        nc.scalar.activation(ksq[:, :, :, :], kp[:, :, :, :], AF.Square)

        if KPHASE < 4:
            continue
        # ---- per-head norm terms: |q_i|^2 broadcast tile, |k_j|^2 cols ----
        qbc, knc = {}, {}
        pqn = {p: psp.tile([128, 256], F32, tag="d2", bufs=2,
                           name=f"pqn{p}") for p in range(2)}
        for l in range(L):
            for p in range(2):
                rg = p * 64
                nc.tensor.matmul(pqn[p][:, :], c["onesb"][rg:rg + 64, 0:128],
                                 qsq[rg:rg + 64, :, l * 8:(l + 1) * 8],
                                 start=(l == 0), stop=(l == L - 1),
                                 tile_position=(rg, 0))
        for p in range(2):
            qbc[p] = pool.tile([128, 256], F32, tag=f"qbc{p}",
                               name=f"qbc{p}")
            if p == 0:
                nc.vector.tensor_copy(qbc[p][:, :], pqn[p][:, :])
            else:
                nc.scalar.activation(qbc[p][:, :], pqn[p][:, :], AF.Copy)
        pkc = {p: psp.tile([128, 2], F32, tag="d2", bufs=2,
                           name=f"pkc{p}") for p in range(2)}
        for jc in range(2):
            for l in range(L):
                for p in range(2):
                    rg = p * 64
                    nc.tensor.matmul(
                        pkc[p][:, jc:jc + 1],
                        ksq[rg:rg + 64, l, jc * 16:(jc + 1) * 16, :],
                        c["onesb"][rg:rg + 64, 0:1],
                        start=(l == 0), stop=(l == L - 1),
                        tile_position=(rg, 0))
        for p in range(2):
            knc[p] = pool.tile([128, 2], F32, tag=f"knc{p}", name=f"knc{p}")
            nc.scalar.activation(knc[p][:, :], pkc[p][:, :], AF.Copy)

        # ---- D2^T chains + pointwise -> attn tiles ----
        attn = {}
        for jc in range(2):
            pd = {p: psp.tile([128, 256], F32, tag="d2", bufs=2,
                               name=f"pd{p}") for p in range(2)}
            for l in range(L):
                for p in range(2):
                    rg = p * 64
                    nc.tensor.matmul(
                        pd[p][:, :],
                        kp[rg:rg + 64, l, jc * 16:(jc + 1) * 16, :],
                        qp[rg:rg + 64, :, l * 8:(l + 1) * 8],
                        start=(l == 0), stop=(l == L - 1),
                        tile_position=(rg, 0))
            for p in range(2):
                tt = pool.tile([128, 256], F32, tag=f"dist{p}", bufs=2)
                af = pool.tile([128, 256], F32, tag=f"af{p}", bufs=2)
                ar = pool.tile([128, 256], BF16, tag=f"ar{p}{jc}")
                nc.vector.tensor_tensor(tt[:, :], pd[p][:, :], qbc[p][:, :],
                                        ALU.add)
                nc.scalar.activation(tt[:, :], tt[:, :], AF.Sqrt,
                                     bias=knc[p][:, jc:jc + 1])
                nc.vector.reciprocal_approx_fast(af[:, :], tt[:, :])
                nc.scalar.activation(ar[:, :], af[:, :], AF.Copy)
                attn[(p, jc)] = ar

        if KPHASE < 5:
            continue
        # ---- V^T via PE transposes (both heads at once) ----
        vT = [pool.tile([128, 2048], BF16, tag=f"vT{jc}", name=f"vT{jc}")
              for jc in range(2)]
        for jc in range(2):
            vTr = vT[jc].rearrange("p (h c l) -> p h l c", h=2, c=64, l=L)
            for lg in range(4):
                pvt = psp.tile([128, 4, 128], BF16, tag="bigr", bufs=2,
                               name="pvt")
                for j in range(4):
                    l = lg * 4 + j
                    nc.tensor.transpose(
                        pvt[:, j, :],
                        vp[:, l, jc * 16:(jc + 1) * 16, :],
                        c["idb"][:, :])
                dst = vTr[:, :, lg * 4:(lg + 1) * 4, :]
                src = pvt[:, :, :].rearrange("p l (h c) -> p h l c", h=2)
                if lg % 2 == 0:
                    nc.vector.tensor_copy(dst, src)
                else:
                    nc.scalar.activation(dst, src, AF.Copy)

        # ---- AV matmuls -> av_pair tiles (i-chunk part, (2h, d) free) ----
        av = [pool.tile([128, 2048], BF16, tag=f"av{ic}", name=f"av{ic}")
              for ic in range(2)]
        for p in range(2):
            for ic in range(2):
                for dh in range(2):
                    pav = psp.tile([128, 512], F32, tag="big", bufs=4)
                    o = p * 1024 + dh * 512
                    for jc in range(2):
                        nc.tensor.matmul(
                            pav[:, :],
                            attn[(p, jc)][:, ic * 128:(ic + 1) * 128],
                            vT[jc][:, o:o + 512],
                            start=(jc == 0), stop=(jc == 1))
                    # av free layout is (l, h, c): d = l*128 + h*64 + c
                    av4 = av[ic].rearrange("p (l h c) -> p h c l", h=2, c=64)
                    dst = av4[:, p, dh * 32:(dh + 1) * 32, :]
                    srcp = pav[:, :].rearrange("p (c l) -> p c l", c=32)
                    if dh == 0:
                        nc.vector.tensor_copy(dst, srcp)
                    else:
                        nc.scalar.activation(dst, srcp, AF.Copy)

        # ---- permute attn-out into merge-conv layout mc_in[hp] ----
        mcr = mc_in[hp][:, :, 1:129].rearrange("p n (l s) -> p l n s", l=L)
        for ic in range(2):
            avT = av[ic].rearrange("p (l h c) -> p l h c", h=2, c=64)
            for lg in range(4):
                pmc = psp.tile([128, 4, 128], BF16, tag="bigr", bufs=2,
                               name="pmc")
                for j in range(4):
                    l = lg * 4 + j
                    nc.tensor.transpose(pmc[:, j, :], avT[:, l, :, :],
                                        c["idb"][:, :])
                dst = mcr[:, lg * 4:(lg + 1) * 4, ic * 16:(ic + 1) * 16, :]
                src = pmc[:, :, :].rearrange("p l (n s) -> p l n s", n=16)
                if lg % 2 == 0:
                    nc.vector.tensor_copy(dst, src)
                else:
                    nc.scalar.activation(dst, src, AF.Copy)

    if KPHASE < 6:
        out_sb = pool.tile([64, N, LT], F32, tag="scr", name="outsb0")
        nc.vector.memset(out_sb[:, :, :], 0.0)
        nc.sync.dma_start(y_d[:, sc * N:(sc + 1) * N, :], out_sb[:, :, :])
        return
    # ---- merge conv + residual -> y2p (padded, duplicated) ----
    xr = pool.tile([128, N, LT], F32, tag="scr")
    nc.sync.dma_start(xr[0:64, :, :], xsrc)
    nc.sync.dma_start(xr[64:128, :, :], xsrc)
    y2p = pool.tile([128, N, LTP], F32, tag="vy")
    for nb in range(NB):
        pm = psp.tile([128, RPB, LT], F32, tag="big", bufs=4)
        for kc in range(4):
            for k in range(3):
                o = (kc * 3 + k) * 128
                nc.tensor.matmul(
                    pm[:, :, :], c["wm"][:, o:o + 128],
                    mc_in[kc][:, nb * RPB:(nb + 1) * RPB, k:k + 128],
                    start=(kc == 0 and k == 0), stop=(kc == 3 and k == 2))
        nc.vector.scalar_tensor_tensor(
            y2p[:, nb * RPB:(nb + 1) * RPB, 1:129], pm[:, :, :],
            bias[:, 12:13], xr[:, nb * RPB:(nb + 1) * RPB, :],
            ALU.add, ALU.add)

    if KPHASE < 7:
        out_sb = pool.tile([64, N, LT], F32, tag="scr", name="outsb0")
        nc.vector.tensor_copy(out_sb[:, :, :], y2p[0:64, :, 1:129])
        nc.sync.dma_start(y_d[:, sc * N:(sc + 1) * N, :], out_sb[:, :, :])
        return
    # ---- GN2 -> z3 (shares z2's slot) ----
    z3 = pool.tile([128, N, LTP], F32R, tag="z")
    nc.gpsimd.memset(z3[:, :, :].bitcast(mybir.dt.uint32), 0)
    _groupnorm(nc, pool, psp, c["gnm"], y2p[:, :, 1:129], z3, "g2",
               c["bias"][:, 16:17])

    # ---- FFN per n-block: ff1 (+swish) then ff2 (+residual) ----
    out_sb = pool.tile([64, N, LT], F32, tag="scr")
    for nb in range(NB):
        h2 = [pool.tile([128, RPB, LTP], F32R, tag=f"h2{mc}", bufs=2,
                        name=f"h2{mc}") for mc in range(2)]
        pfs = [psp.tile([128, RPB, LT], F32, tag="big", bufs=4,
                        name=f"pf{i}") for i in range(2)]
        for mc in range(2):
            nc.gpsimd.memset(h2[mc][:, :, :].bitcast(mybir.dt.uint32), 0)
        _conv_k64_pair(nc, [
            (pfs[0], c["wf1"], lambda k: k * 256, z3, nb, 0),
            (pfs[1], c["wf1"], lambda k: k * 256 + 128, z3, nb, 64)])
        for mc in range(2):
            pf = pfs[mc]
            sg = pool.tile([128, RPB, LT], F32, tag="sg", bufs=2, name="sg")
            nc.scalar.activation(sg[:, :, :], pf[:, :, :], AF.Sigmoid,
                                 bias=bias[:, 13 + mc:14 + mc])
            nc.vector.scalar_tensor_tensor(h2[mc][:, :, 1:129], pf[:, :, :],
                                           bias[:, 13 + mc:14 + mc],
                                           sg[:, :, :], ALU.add, ALU.mult)
        po = psp.tile([128, RPB, LT], F32, tag="big", bufs=4)
        for kc in range(2):
            for k in range(3):
                o = (kc * 3 + k) * 128
                nc.tensor.matmul(po[:, :, :], c["wf2"][:, o:o + 128],
                                 h2[kc][:, :, k:k + 128],
                                 start=(kc == 0 and k == 0),
                                 stop=(kc == 1 and k == 2))
        nc.vector.scalar_tensor_tensor(
            out_sb[:, nb * RPB:(nb + 1) * RPB, :], po[0:64, :, :],
            bias[0:64, 15:16], y2p[0:64, nb * RPB:(nb + 1) * RPB, 1:129],
            ALU.add, ALU.add)
    nc.sync.dma_start(y_d[:, sc * N:(sc + 1) * N, :], out_sb[:, :, :])


# ---------------------------------------------------------------------------
# host side
# ---------------------------------------------------------------------------

def _prep_consts(inputs):
    f = np.float32
    gn1_g = np.asarray(inputs["gn1_g"], f)
    gn1_b = np.asarray(inputs["gn1_b"], f)
    gn2_g = np.asarray(inputs["gn2_g"], f)
    gn2_b = np.asarray(inputs["gn2_b"], f)
    w_qkv = np.asarray(inputs["w_qkv"], f)      # (1536, 64, 3)
    b_qkv = np.asarray(inputs["b_qkv"], f)
    w_merge = np.asarray(inputs["w_merge"], f)  # (64, 512, 3)
    b_merge = np.asarray(inputs["b_merge"], f)
    w_ff1 = np.asarray(inputs["w_ff1"], f)      # (256, 64, 3)
    b_ff1 = np.asarray(inputs["b_ff1"], f)
    w_ff2 = np.asarray(inputs["w_ff2"], f)      # (64, 256, 3)
    b_ff2 = np.asarray(inputs["b_ff2"], f)

    co_scale = np.ones((1536,), f)
    co_scale[0:512] = -2.0  # q-branch
    wq_eff = w_qkv * gn1_g[None, :, None] * co_scale[:, None, None]
    wqT = np.ascontiguousarray(wq_eff.transpose(1, 2, 0)).reshape(64, 3 * 1536)
    wq_host = np.concatenate([wqT, wqT], axis=0)  # (128, 4608)

    Ck = np.einsum("oik,i->ok", w_qkv, gn1_b) * co_scale[:, None]
    assert abs(Ck).max() == 0.0, "nonzero gn1 beta needs boundary fixups"
    bq_eff = b_qkv * co_scale + Ck.sum(1)

    wmT = np.ascontiguousarray(w_merge.transpose(1, 2, 0))  # (512, 3, 64)
    wm_host = np.concatenate([wmT, wmT], axis=2)            # (512, 3, 128)
    wm_host = wm_host.reshape(4, 128, 3 * 128).transpose(1, 0, 2) \
                     .reshape(128, 4 * 3 * 128)

    wf1_eff = w_ff1 * gn2_g[None, :, None]
    Ck2 = np.einsum("oik,i->ok", w_ff1, gn2_b)
    assert abs(Ck2).max() == 0.0, "nonzero gn2 beta needs boundary fixups"
    bf1_eff = b_ff1 + Ck2.sum(1)
    wf1T = np.ascontiguousarray(wf1_eff.transpose(1, 2, 0)).reshape(64, 768)
    wf1_host = np.concatenate([wf1T, wf1T], axis=0)  # (128, 768)

    wf2T = np.ascontiguousarray(w_ff2.transpose(1, 2, 0))  # (256, 3, 64)
    wf2_host = np.concatenate([wf2T, wf2T], axis=2)        # (256, 3, 128)
    wf2_host = wf2_host.reshape(2, 128, 3 * 128).transpose(1, 0, 2) \
                       .reshape(128, 2 * 3 * 128)

    biases = np.zeros((128, 17), f)
    biases[:, 16] = EPS
    biases[:, 0:12] = bq_eff.reshape(12, 128).T
    biases[:, 12] = np.concatenate([b_merge, b_merge])
    biases[:, 13:15] = bf1_eff.reshape(2, 128).T
    biases[:, 15] = np.concatenate([b_ff2, b_ff2])

    p = np.arange(128)
    gnm = ((p[:, None] % 64) // 8 == (p[None, :] % 64) // 8).astype(f)
    gnm /= (D // GROUPS) * LT * 2  # mean over group, duplicated halves

    return {
        "wq": np.ascontiguousarray(wq_host),
        "wm": np.ascontiguousarray(wm_host).astype(ml_dtypes.bfloat16),
        "wf1": np.ascontiguousarray(wf1_host),
        "wf2": np.ascontiguousarray(wf2_host),
        "biases": biases,
        "gnm": gnm,
        "onesb": np.ones((128, 256), ml_dtypes.bfloat16),
        "idb": np.eye(128, dtype=f).astype(ml_dtypes.bfloat16),
    }


def build_in_maps(inputs):
    x = np.ascontiguousarray(np.asarray(inputs["x"], np.float32))
    consts = _prep_consts(inputs)
    in_maps = []
    for core in range(NCORES):
        m = dict(consts)
        m["x"] = np.ascontiguousarray(
            x[core * ROWS:(core + 1) * ROWS].transpose(1, 0, 2))
        in_maps.append(m)
    return in_maps


def get_program():
    if "nc" not in _CACHE:
        _CACHE["nc"] = _build()
    return _CACHE["nc"]


def kernel(**inputs) -> np.ndarray:
    nc = get_program()
    in_maps = build_in_maps(inputs)
    res = bass_utils.run_bass_kernel_spmd(nc, in_maps,
                                          core_ids=list(range(NCORES)))
    out = np.concatenate(
        [res.results[c]["y"].transpose(1, 0, 2) for c in range(NCORES)],
        axis=0)
    return np.ascontiguousarray(out).astype(np.float32)
